# revision 17
# baseline (speedup 1.0000x reference)
"""Causal ReLU-attention block (qkv proj + per-head attention) on 8 trn2 cores.

Sharding: pure data-parallel over batch (B=8 -> 1 batch element per core).

Per-core structure (fused single pass, PE-stream order):
  warm-up MMs -> qk ots for head-pair group 0 -> v slice 0 ->
  [attention group 0 interleaved with qk/v chains for group 1] ->
  [attention group 1 interleaved with qk/v chains for group 2] ->
  attention group 2.

DMA: host packs x/W into per-partition-contiguous blocks ordered by first
use; sync ring carries x (+ y out), scalar ring carries W/biases, with W
issue instructions interleaved between ACT evictions.
"""

import sys
from collections import deque
from contextlib import ExitStack

sys.path.insert(0, "/opt/trn_rl_repo")

import ml_dtypes
import numpy as np

import concourse.bass as bass
import concourse.tile as tile
from concourse import bacc, bass_utils, mybir

P = 128
QW = 512  # t_q chunk width (PSUM bank = 512 fp32)

BF16 = mybir.dt.bfloat16
F32 = mybir.dt.float32
AF = mybir.ActivationFunctionType
ALU = mybir.AluOpType


def build_module(T=1024, C=768, H=12, n_cores=8):
    """Build + compile the per-core Bass module (same program on all cores)."""
    hd = C // H
    assert hd == 64 and H % 2 == 0 and C % P == 0 and T % QW == 0
    CT = C // P            # contraction tiles over C (6)
    TT = T // P            # t tiles (8)
    NQC = T // QW          # q chunks (2)
    NHP = H // 2           # head pairs (6)
    NG = NHP // 2          # attention groups of 2 head-pair streams (3)
    scale = 1.0 / float(np.sqrt(hd))

    nc = bacc.Bacc("TRN2", target_bir_lowering=False, debug=False,
                   num_devices=n_cores)

    # host-packed inputs: per-partition-contiguous blocks in use order
    #  xd: [p, 3 ct-pair blocks, 2 ct, T]
    #  wd: [p, 9 blocks, ct, 256] with blocks
    #      [q01, k01, v0, q23, k23, v1, q45, k45, v2]
    xd = nc.dram_tensor("xd", [P, CT, T], BF16, kind="ExternalInput").ap()
    wd = nc.dram_tensor("wd", [P, 9, CT, 256], BF16,
                        kind="ExternalInput").ap()
    bqk = nc.dram_tensor("bqk", [P, 2 * CT], F32, kind="ExternalInput").ap()
    bvr = nc.dram_tensor("bvr", [1, C], BF16, kind="ExternalInput").ap()
    yT = nc.dram_tensor("yT", [C, T], F32, kind="ExternalOutput").ap()

    with tile.TileContext(nc) as tc, ExitStack() as ctx:
        const = ctx.enter_context(tc.tile_pool(name="const", bufs=1))
        # PSUM: "s" 4 banks (per-head score tiles), "f" 2 banks (qk/v
        # chains), "y" 2 banks (attention accumulators) = 8 banks total
        spsum = ctx.enter_context(tc.tile_pool(name="spsum", bufs=4, space="PSUM"))
        fpsum = ctx.enter_context(tc.tile_pool(name="fpsum", bufs=2, space="PSUM"))
        ypsum = ctx.enter_context(tc.tile_pool(name="ypsum", bufs=2, space="PSUM"))
        scb = ctx.enter_context(tc.tile_pool(name="scb", bufs=14))
        ysb = ctx.enter_context(tc.tile_pool(name="ysb", bufs=3))

        wt_sb = const.tile([P, 9, CT, 256], BF16)
        xt_sb = const.tile([P, CT, T], BF16)
        bqk_sb = const.tile([P, 2 * CT], F32)
        bvr_sb = const.tile([1, C], BF16)
        ones_sb = const.tile([1, P], BF16)
        qkT = const.tile([P, 2 * CT, T], BF16)   # o-tiles: q = 0..5, k = 6..11
        vsb = const.tile([P, TT, C], BF16)       # v in natural [t, o] layout
        mask_sb = const.tile([P, QW], BF16)

        # ---- input DMA issue (ring order == consumption order) ----
        # Split x per-ct across both rings; first W blocks lead on sync.
        # Remaining W blocks are interleaved between ACT evictions below.
        nc.sync.dma_start(wt_sb[:, 0], wd[:, 0])     # Wq01
        nc.scalar.dma_start(bqk_sb[:], bqk[:])
        nc.scalar.dma_start(bvr_sb[:], bvr[:])
        for ct in range(CT):
            eng = nc.sync if ct % 2 == 0 else nc.scalar
            eng.dma_start(xt_sb[:, ct, :], xd[:, ct])
        nc.sync.dma_start(wt_sb[:, 1], wd[:, 1])     # Wk01
        w_dma = deque(range(2, 9))                   # v0, q23, k23, v1, ...

        def issue_w():
            if w_dma:
                b = w_dma.popleft()
                nc.scalar.dma_start(wt_sb[:, b], wd[:, b])

        # 0/1 upper-triangle mask const (also used as warm-up operand)
        nc.gpsimd.memset(ones_sb[:], 1.0)
        nc.gpsimd.memset(mask_sb[:], 1.0)
        nc.gpsimd.affine_select(
            mask_sb[:], mask_sb[:], pattern=[[1, QW]],
            compare_op=ALU.is_ge, fill=0.0, base=0, channel_multiplier=-1)

        # ---- PE warm-up: keep HAM busy while first inputs stream in ----
        warm_ps = ypsum.tile([P, QW], F32, tag="y", name="warm")
        for _ in range(6):
            nc.tensor.matmul(warm_ps[:], mask_sb[:, 0:P], mask_sb[:],
                             start=True, stop=True)

        evict = [0]

        def emit_qk(ot):
            # ot 0..5 = q features (head pair = ot), 6..11 = k features;
            # one 1-bank psum tile + one ACT bias-evict per q chunk
            j = ot if ot < CT else ot - CT
            blk = 3 * (j // 2) + (0 if ot < CT else 1)
            off = (j % 2) * P
            for qc in range(NQC):
                ps = fpsum.tile([P, QW], F32, tag="f", name="qk_ps")
                for ct in range(CT):
                    nc.tensor.matmul(
                        ps[:],
                        wt_sb[:, blk, ct, off:off + P],
                        xt_sb[:, ct, qc * QW:(qc + 1) * QW],
                        start=(ct == 0), stop=(ct == CT - 1),
                    )
                if qc == 0:
                    issue_w()
                nc.scalar.activation(
                    qkT[:, ot, qc * QW:(qc + 1) * QW], ps[:],
                    AF.Identity, bias=bqk_sb[:, ot:ot + 1])

        def emit_v(s, tp):
            # v features s*256..(s+1)*256 for t-tiles 2tp, 2tp+1; the bias
            # lands via a final K=1 rank-1 matmul (ones^T @ bias_row)
            ps = fpsum.tile([P, QW], F32, tag="f", name="v_ps")
            for j in range(2):
                tt = 2 * tp + j
                for ct in range(CT):
                    nc.tensor.matmul(
                        ps[:, j * 256:(j + 1) * 256],
                        xt_sb[:, ct, tt * P:(tt + 1) * P],
                        wt_sb[:, 3 * s + 2, ct, :],
                        start=(ct == 0), stop=False,
                    )
                nc.tensor.matmul(
                    ps[:, j * 256:(j + 1) * 256], ones_sb[:],
                    bvr_sb[:, s * 256:(s + 1) * 256],
                    start=False, stop=True,
                )
            dst = vsb[:, 2 * tp:2 * tp + 2, s * 256:(s + 1) * 256]
            src = ps.rearrange("p (a b) -> p a b", a=2)
            if evict[0] % 2 == 0:
                nc.scalar.activation(dst, src, AF.Copy)
            else:
                nc.vector.tensor_copy(dst, src)
            evict[0] += 1

        def relu_evict(dst, src, h):
            # relu(scale * s) per head: PSUM -> SBUF bf16, h0 on ACT and h1
            # on DVE so both halves evict in parallel
            if h == 0:
                nc.scalar.activation(dst, src, AF.Relu, scale=scale)
            else:
                nc.vector.tensor_scalar(dst, src, scale, 0.0, ALU.mult, ALU.max)

        def attention_closures(hp):
            """Per-item (scores, att@v) emission closures for one head pair;
            interleaver runs att@v LAG items behind scores."""
            items = []
            for qc in range(NQC):
                kb_hi = min((qc * QW + QW - 1) // P, TT - 1)
                for kb in range(kb_hi + 1):
                    items.append((qc, kb, kb_hi))
            state = {"s": {}, "y": {}}
            sc_fns, av_fns = [], []

            def sc(i, qc, kb, kb_hi):
                delta = max(kb * P - qc * QW, 0)   # first valid t_q col
                sp = [spsum.tile([P, QW], F32, tag="s", name="s_ps")
                      for _ in range(2)]
                for h, ppos in ((0, (0, 0)), (1, (64, 0))):
                    nc.tensor.matmul(
                        sp[h][:, delta:QW],
                        qkT[h * 64:(h + 1) * 64, CT + hp,
                            kb * P:(kb + 1) * P],
                        qkT[h * 64:(h + 1) * 64, hp,
                            qc * QW + delta:(qc + 1) * QW],
                        start=True, stop=True, tile_position=ppos,
                    )
                s = scb.tile([P, 2, QW], BF16, tag="s")
                for h in range(2):
                    relu_evict(s[:, h, delta:QW], sp[h][:, delta:QW], h)
                if kb * P >= qc * QW:   # diagonal block: causal mask on the
                    # first P cols only (row p can only mask j' < p < P)
                    nc.gpsimd.affine_select(
                        s[:, :, delta:delta + P],
                        s[:, :, delta:delta + P],
                        pattern=[[0, 2], [1, P]],
                        compare_op=ALU.is_ge, fill=0.0,
                        base=0, channel_multiplier=-1,
                    )
                state["s"][i] = s

            def av(i, qc, kb, kb_hi):
                if kb == 0:
                    state["y"][qc] = ypsum.tile([P, QW], F32, tag="y",
                                                name="yp")
                yp = state["y"][qc]
                delta = max(kb * P - qc * QW, 0)
                s = state["s"].pop(i)
                # two heads accumulate into disjoint partition ranges of one
                # bank; each runs its own start/stop group
                nc.tensor.matmul(
                    yp[0:64, delta:QW], vsb[:, kb, hp * P:hp * P + 64],
                    s[:, 0, delta:QW],
                    start=(kb == 0), stop=(kb == kb_hi),
                    tile_position=(0, 0), skip_group_check=True,
                )
                nc.tensor.matmul(
                    yp[64:128, delta:QW],
                    vsb[:, kb, hp * P + 64:hp * P + 128],
                    s[:, 1, delta:QW],
                    start=(kb == 0), stop=(kb == kb_hi),
                    tile_position=(0, 64), skip_group_check=True,
                )
                if kb == kb_hi:
                    yp = state["y"].pop(qc)
                    yt = ysb.tile([P, QW], F32, tag="yt")
                    nc.vector.tensor_copy(yt[:], yp[:])
                    nc.sync.dma_start(
                        yT[hp * P:(hp + 1) * P, qc * QW:(qc + 1) * QW],
                        yt[:])

            for i, (qc, kb, kb_hi) in enumerate(items):
                sc_fns.append(
                    lambda i=i, qc=qc, kb=kb, kb_hi=kb_hi: sc(i, qc, kb, kb_hi))
                av_fns.append(
                    lambda i=i, qc=qc, kb=kb, kb_hi=kb_hi: av(i, qc, kb, kb_hi))
            return sc_fns, av_fns

        def group_fillers(g):
            """qk + v chain thunks needed by attention group g."""
            fns = []
            for hp in (2 * g, 2 * g + 1):
                fns.append(lambda ot=hp: emit_qk(ot))
                fns.append(lambda ot=CT + hp: emit_qk(ot))
            for tp in range(TT // 2):
                fns.append(lambda s=g, tp=tp: emit_v(s, tp))
            return fns

        # ---- fused emission ----
        # pre-phase: everything group 0 needs
        for fn in group_fillers(0):
            fn()

        LAG = 3
        for g in range(NG):
            streams = [attention_closures(hp) for hp in (2 * g, 2 * g + 1)]
            fillers = deque(group_fillers(g + 1) if g + 1 < NG else [])
            n = len(streams[0][0])
            for i in range(n + LAG):
                if i < n:
                    for sc_fns, _ in streams:
                        sc_fns[i]()
                if i >= LAG:
                    for _, av_fns in streams:
                        av_fns[i - LAG]()
                if fillers:
                    fillers.popleft()()
                    # 12 fillers over 12 steps; drain stragglers at the end
                    if i == n + LAG - 1:
                        while fillers:
                            fillers.popleft()()

    nc.compile()
    return nc


_CACHE = {}


def _get_module():
    if "nc" not in _CACHE:
        _CACHE["nc"] = build_module()
    return _CACHE["nc"]


def _prep_in_maps(x, W_attn, b_attn, T=1024, C=768, n_cores=8):
    bf = ml_dtypes.bfloat16
    CT = C // P
    WT = np.ascontiguousarray(W_attn.astype(np.float32).T)     # [C, 3C]
    # [C, 3C] -> [p, ct, 3C]
    W3 = WT.reshape(CT, P, 3 * C).transpose(1, 0, 2)
    # 9 blocks of 256 o-cols in use order [q01,k01,v0,q23,k23,v1,q45,k45,v2]
    blocks = []
    for gg in range(3):
        blocks.append(W3[:, :, gg * 256:(gg + 1) * 256])             # q
        blocks.append(W3[:, :, C + gg * 256:C + (gg + 1) * 256])     # k
        blocks.append(W3[:, :, 2 * C + gg * 256:2 * C + (gg + 1) * 256])  # v
    wd = np.ascontiguousarray(
        np.stack(blocks, axis=1)).astype(bf)                   # [p, 9, ct, 256]
    bqk = np.ascontiguousarray(
        b_attn[:2 * C].astype(np.float32).reshape(2 * CT, P).T)  # [P, 12]
    bvr = np.ascontiguousarray(
        b_attn[2 * C:].astype(np.float32)[None, :]).astype(bf)  # [1, C]
    in_maps = []
    for c in range(n_cores):
        xT_b = np.ascontiguousarray(x[c].astype(np.float32).T)  # [C, T]
        xdb = np.ascontiguousarray(
            xT_b.reshape(CT, P, T).transpose(1, 0, 2)).astype(bf)
        in_maps.append({"xd": xdb, "wd": wd, "bqk": bqk, "bvr": bvr})
    return in_maps


def run(x, W_attn, b_attn, trace=False):
    nc = _get_module()
    in_maps = _prep_in_maps(x, W_attn, b_attn)
    res = bass_utils.run_bass_kernel_spmd(
        nc, in_maps, core_ids=list(range(8)), trace=trace)
    y = np.stack([np.asarray(res.results[c]["yT"]).T for c in range(8)])
    return np.ascontiguousarray(y.astype(np.float32)), res


def kernel(x, W_attn, b_attn):
    y, _ = run(x, W_attn, b_attn, trace=False)
    return y


# revision 20
# speedup vs baseline: 1.0232x; 1.0232x over previous
"""Causal ReLU-attention block (qkv proj + per-head attention) on 8 trn2 cores.

Sharding: pure data-parallel over batch (B=8 -> 1 batch element per core).

Per-core structure (fused single pass, PE-stream order):
  warm-up MMs -> qk ots for head-pair group 0 -> v slice 0 ->
  [attention group 0 interleaved with qk/v chains for group 1] ->
  [attention group 1 interleaved with qk/v chains for group 2] ->
  attention group 2.

DMA: host packs x/W into per-partition-contiguous blocks ordered by first
use; sync ring carries x (+ y out), scalar ring carries W/biases, with W
issue instructions interleaved between ACT evictions.
"""

import sys
from collections import deque
from contextlib import ExitStack

sys.path.insert(0, "/opt/trn_rl_repo")

import ml_dtypes
import numpy as np

import concourse.bass as bass
import concourse.tile as tile
from concourse import bacc, bass_utils, mybir

P = 128
QW = 512  # t_q chunk width (PSUM bank = 512 fp32)

BF16 = mybir.dt.bfloat16
F32 = mybir.dt.float32
AF = mybir.ActivationFunctionType
ALU = mybir.AluOpType


def build_module(T=1024, C=768, H=12, n_cores=8):
    """Build + compile the per-core Bass module (same program on all cores)."""
    hd = C // H
    assert hd == 64 and H % 2 == 0 and C % P == 0 and T % QW == 0
    CT = C // P            # contraction tiles over C (6)
    TT = T // P            # t tiles (8)
    NQC = T // QW          # q chunks (2)
    NHP = H // 2           # head pairs (6)
    NG = NHP // 2          # attention groups of 2 head-pair streams (3)
    scale = 1.0 / float(np.sqrt(hd))

    nc = bacc.Bacc("TRN2", target_bir_lowering=False, debug=False,
                   num_devices=n_cores)

    # host-packed inputs: per-partition-contiguous blocks in use order
    #  xd: [p, 3 ct-pair blocks, 2 ct, T]
    #  wd: [p, 9 blocks, ct, 256] with blocks
    #      [q01, k01, v0, q23, k23, v1, q45, k45, v2]
    xd = nc.dram_tensor("xd", [P, CT, T], BF16, kind="ExternalInput").ap()
    wd = nc.dram_tensor("wd", [P, 9, CT, 256], BF16,
                        kind="ExternalInput").ap()
    bqk = nc.dram_tensor("bqk", [P, 2 * CT], F32, kind="ExternalInput").ap()
    bvr = nc.dram_tensor("bvr", [1, C], BF16, kind="ExternalInput").ap()
    yT = nc.dram_tensor("yT", [C, T], F32, kind="ExternalOutput").ap()

    with tile.TileContext(nc) as tc, ExitStack() as ctx:
        const = ctx.enter_context(tc.tile_pool(name="const", bufs=1))
        # PSUM: "s" 4 banks (score tiles), "f" 2 banks (qk/v chains),
        # "y" 2 banks (attention accumulators) = 8 banks total
        spsum = ctx.enter_context(tc.tile_pool(name="spsum", bufs=2, space="PSUM"))
        fpsum = ctx.enter_context(tc.tile_pool(name="fpsum", bufs=2, space="PSUM"))
        ypsum = ctx.enter_context(tc.tile_pool(name="ypsum", bufs=2, space="PSUM"))
        scb = ctx.enter_context(tc.tile_pool(name="scb", bufs=14))
        ysb = ctx.enter_context(tc.tile_pool(name="ysb", bufs=3))

        wt_sb = const.tile([P, 9, CT, 256], BF16)
        xt_sb = const.tile([P, CT, T], BF16)
        bqk_sb = const.tile([P, 2 * CT], F32)
        bvr_sb = const.tile([1, C], BF16)
        ones_sb = const.tile([1, P], BF16)
        qkT = const.tile([P, 2 * CT, T], BF16)   # o-tiles: q = 0..5, k = 6..11
        vsb = const.tile([P, TT, C], BF16)       # v in natural [t, o] layout
        mask_sb = const.tile([P, QW], BF16)

        # ---- input DMA issue (ring order == consumption order) ----
        # Split x per-ct across both rings; first W blocks lead on sync.
        # Remaining W blocks are interleaved between ACT evictions below.
        nc.sync.dma_start(wt_sb[:, 0], wd[:, 0])     # Wq01
        nc.scalar.dma_start(bqk_sb[:], bqk[:])
        nc.scalar.dma_start(bvr_sb[:], bvr[:])
        for ct in range(CT):
            eng = nc.sync if ct % 2 == 0 else nc.scalar
            eng.dma_start(xt_sb[:, ct, :], xd[:, ct])
        nc.sync.dma_start(wt_sb[:, 1], wd[:, 1])     # Wk01
        w_dma = deque(range(2, 9))                   # v0, q23, k23, v1, ...

        def issue_w():
            if w_dma:
                b = w_dma.popleft()
                nc.scalar.dma_start(wt_sb[:, b], wd[:, b])

        # 0/1 upper-triangle mask const (also used as warm-up operand)
        nc.gpsimd.memset(ones_sb[:], 1.0)
        nc.gpsimd.memset(mask_sb[:], 1.0)
        nc.gpsimd.affine_select(
            mask_sb[:], mask_sb[:], pattern=[[1, QW]],
            compare_op=ALU.is_ge, fill=0.0, base=0, channel_multiplier=-1)

        # ---- PE warm-up: keep HAM busy while first inputs stream in ----
        warm_ps = ypsum.tile([P, QW], F32, tag="y", name="warm")
        for _ in range(6):
            nc.tensor.matmul(warm_ps[:], mask_sb[:, 0:P], mask_sb[:],
                             start=True, stop=True)

        evict = [0]

        def emit_qk(ot):
            # ot 0..5 = q features (head pair = ot), 6..11 = k features;
            # one 1-bank psum tile + one ACT bias-evict per q chunk
            j = ot if ot < CT else ot - CT
            blk = 3 * (j // 2) + (0 if ot < CT else 1)
            off = (j % 2) * P
            for qc in range(NQC):
                ps = fpsum.tile([P, QW], F32, tag="f", name="qk_ps")
                for ct in range(CT):
                    nc.tensor.matmul(
                        ps[:],
                        wt_sb[:, blk, ct, off:off + P],
                        xt_sb[:, ct, qc * QW:(qc + 1) * QW],
                        start=(ct == 0), stop=(ct == CT - 1),
                    )
                if qc == 0:
                    issue_w()
                nc.scalar.activation(
                    qkT[:, ot, qc * QW:(qc + 1) * QW], ps[:],
                    AF.Identity, bias=bqk_sb[:, ot:ot + 1])

        def emit_v(s, tp):
            # v features s*256..(s+1)*256 for t-tiles 2tp, 2tp+1; the bias
            # lands via a final K=1 rank-1 matmul (ones^T @ bias_row)
            ps = fpsum.tile([P, QW], F32, tag="f", name="v_ps")
            for j in range(2):
                tt = 2 * tp + j
                for ct in range(CT):
                    nc.tensor.matmul(
                        ps[:, j * 256:(j + 1) * 256],
                        xt_sb[:, ct, tt * P:(tt + 1) * P],
                        wt_sb[:, 3 * s + 2, ct, :],
                        start=(ct == 0), stop=False,
                    )
                nc.tensor.matmul(
                    ps[:, j * 256:(j + 1) * 256], ones_sb[:],
                    bvr_sb[:, s * 256:(s + 1) * 256],
                    start=False, stop=True,
                )
            dst = vsb[:, 2 * tp:2 * tp + 2, s * 256:(s + 1) * 256]
            src = ps.rearrange("p (a b) -> p a b", a=2)
            if evict[0] % 2 == 0:
                nc.scalar.activation(dst, src, AF.Copy)
            else:
                nc.vector.tensor_copy(dst, src)
            evict[0] += 1

        def relu_evict(dst, src):
            # relu(scale * s): PSUM -> SBUF bf16, alternating ACT / DVE per
            # item (one instruction per item keeps the av pair in sync)
            if evict[0] % 2 == 0:
                nc.scalar.activation(dst, src, AF.Relu, scale=scale)
            else:
                nc.vector.tensor_scalar(dst, src, scale, 0.0, ALU.mult, ALU.max)
            evict[0] += 1

        def attention_closures(hp):
            """Per-item (scores, att@v) emission closures for one head pair;
            interleaver runs att@v LAG items behind scores."""
            items = []
            for qc in range(NQC):
                kb_hi = min((qc * QW + QW - 1) // P, TT - 1)
                for kb in range(kb_hi + 1):
                    items.append((qc, kb, kb_hi))
            state = {"s": {}, "y": {}}
            sc_fns, av_fns = [], []

            def sc(i, qc, kb, kb_hi):
                delta = max(kb * P - qc * QW, 0)   # first valid t_q col
                sp = spsum.tile([P, 2, QW], F32, tag="s", name="s_ps")
                for h, ppos in ((0, (0, 0)), (1, (64, 0))):
                    nc.tensor.matmul(
                        sp[:, h, delta:QW],
                        qkT[h * 64:(h + 1) * 64, CT + hp,
                            kb * P:(kb + 1) * P],
                        qkT[h * 64:(h + 1) * 64, hp,
                            qc * QW + delta:(qc + 1) * QW],
                        start=True, stop=True, tile_position=ppos,
                    )
                s = scb.tile([P, 2, QW], BF16, tag="s")
                relu_evict(s[:, :, delta:QW], sp[:, :, delta:QW])
                if kb * P >= qc * QW:   # diagonal block: causal mask on the
                    # first P cols only (row p can only mask j' < p < P)
                    nc.gpsimd.affine_select(
                        s[:, :, delta:delta + P],
                        s[:, :, delta:delta + P],
                        pattern=[[0, 2], [1, P]],
                        compare_op=ALU.is_ge, fill=0.0,
                        base=0, channel_multiplier=-1,
                    )
                state["s"][i] = s

            def av(i, qc, kb, kb_hi):
                if kb == 0:
                    state["y"][qc] = ypsum.tile([P, QW], F32, tag="y",
                                                name="yp")
                yp = state["y"][qc]
                delta = max(kb * P - qc * QW, 0)
                s = state["s"].pop(i)
                # two heads accumulate into disjoint partition ranges of one
                # bank; each runs its own start/stop group
                nc.tensor.matmul(
                    yp[0:64, delta:QW], vsb[:, kb, hp * P:hp * P + 64],
                    s[:, 0, delta:QW],
                    start=(kb == 0), stop=(kb == kb_hi),
                    tile_position=(0, 0), skip_group_check=True,
                )
                nc.tensor.matmul(
                    yp[64:128, delta:QW],
                    vsb[:, kb, hp * P + 64:hp * P + 128],
                    s[:, 1, delta:QW],
                    start=(kb == 0), stop=(kb == kb_hi),
                    tile_position=(0, 64), skip_group_check=True,
                )
                if kb == kb_hi:
                    yp = state["y"].pop(qc)
                    yt = ysb.tile([P, QW], F32, tag="yt")
                    nc.vector.tensor_copy(yt[:], yp[:])
                    nc.sync.dma_start(
                        yT[hp * P:(hp + 1) * P, qc * QW:(qc + 1) * QW],
                        yt[:])

            for i, (qc, kb, kb_hi) in enumerate(items):
                sc_fns.append(
                    lambda i=i, qc=qc, kb=kb, kb_hi=kb_hi: sc(i, qc, kb, kb_hi))
                av_fns.append(
                    lambda i=i, qc=qc, kb=kb, kb_hi=kb_hi: av(i, qc, kb, kb_hi))
            return sc_fns, av_fns

        def group_fillers(g):
            """qk + v chain thunks needed by attention group g."""
            fns = []
            for hp in (2 * g, 2 * g + 1):
                fns.append(lambda ot=hp: emit_qk(ot))
                fns.append(lambda ot=CT + hp: emit_qk(ot))
            for tp in range(TT // 2):
                fns.append(lambda s=g, tp=tp: emit_v(s, tp))
            return fns

        # ---- fused emission ----
        # pre-phase: everything group 0 needs
        for fn in group_fillers(0):
            fn()

        LAG = 3
        for g in range(NG):
            streams = [attention_closures(hp) for hp in (2 * g, 2 * g + 1)]
            fillers = deque(group_fillers(g + 1) if g + 1 < NG else [])
            n = len(streams[0][0])
            for i in range(n + LAG):
                if i < n:
                    for sc_fns, _ in streams:
                        sc_fns[i]()
                if i >= LAG:
                    for _, av_fns in streams:
                        av_fns[i - LAG]()
                if fillers:
                    fillers.popleft()()
                    # 12 fillers over 12 steps; drain stragglers at the end
                    if i == n + LAG - 1:
                        while fillers:
                            fillers.popleft()()

    nc.compile()
    return nc


_CACHE = {}


def _get_module():
    if "nc" not in _CACHE:
        _CACHE["nc"] = build_module()
    return _CACHE["nc"]


def _prep_in_maps(x, W_attn, b_attn, T=1024, C=768, n_cores=8):
    bf = ml_dtypes.bfloat16
    CT = C // P
    WT = np.ascontiguousarray(W_attn.astype(np.float32).T)     # [C, 3C]
    # [C, 3C] -> [p, ct, 3C]
    W3 = WT.reshape(CT, P, 3 * C).transpose(1, 0, 2)
    # 9 blocks of 256 o-cols in use order [q01,k01,v0,q23,k23,v1,q45,k45,v2]
    blocks = []
    for gg in range(3):
        blocks.append(W3[:, :, gg * 256:(gg + 1) * 256])             # q
        blocks.append(W3[:, :, C + gg * 256:C + (gg + 1) * 256])     # k
        blocks.append(W3[:, :, 2 * C + gg * 256:2 * C + (gg + 1) * 256])  # v
    wd = np.ascontiguousarray(
        np.stack(blocks, axis=1)).astype(bf)                   # [p, 9, ct, 256]
    bqk = np.ascontiguousarray(
        b_attn[:2 * C].astype(np.float32).reshape(2 * CT, P).T)  # [P, 12]
    bvr = np.ascontiguousarray(
        b_attn[2 * C:].astype(np.float32)[None, :]).astype(bf)  # [1, C]
    in_maps = []
    for c in range(n_cores):
        xT_b = np.ascontiguousarray(x[c].astype(np.float32).T)  # [C, T]
        xdb = np.ascontiguousarray(
            xT_b.reshape(CT, P, T).transpose(1, 0, 2)).astype(bf)
        in_maps.append({"xd": xdb, "wd": wd, "bqk": bqk, "bvr": bvr})
    return in_maps


def run(x, W_attn, b_attn, trace=False):
    nc = _get_module()
    in_maps = _prep_in_maps(x, W_attn, b_attn)
    res = bass_utils.run_bass_kernel_spmd(
        nc, in_maps, core_ids=list(range(8)), trace=trace)
    y = np.stack([np.asarray(res.results[c]["yT"]).T for c in range(8)])
    return np.ascontiguousarray(y.astype(np.float32)), res


def kernel(x, W_attn, b_attn):
    y, _ = run(x, W_attn, b_attn, trace=False)
    return y


# revision 24
# speedup vs baseline: 1.0388x; 1.0152x over previous
"""Causal ReLU-attention block (qkv proj + per-head attention) on 8 trn2 cores.

Sharding: pure data-parallel over batch (B=8 -> 1 batch element per core).

Per-core structure (fused single pass, PE-stream order):
  warm-up MMs -> qk ots for head-pair group 0 -> v slice 0 ->
  [attention group 0 interleaved with qk/v chains for group 1] ->
  [attention group 1 interleaved with qk/v chains for group 2] ->
  attention group 2.

DMA: host packs x/W into per-partition-contiguous blocks ordered by first
use; sync ring carries x (+ y out), scalar ring carries W/biases, with W
issue instructions interleaved between ACT evictions.
"""

import sys
from collections import deque
from contextlib import ExitStack

sys.path.insert(0, "/opt/trn_rl_repo")

import ml_dtypes
import numpy as np

import concourse.bass as bass
import concourse.tile as tile
from concourse import bacc, bass_utils, mybir

P = 128
QW = 512  # t_q chunk width (PSUM bank = 512 fp32)

BF16 = mybir.dt.bfloat16
F32 = mybir.dt.float32
AF = mybir.ActivationFunctionType
ALU = mybir.AluOpType


def build_module(T=1024, C=768, H=12, n_cores=8):
    """Build + compile the per-core Bass module (same program on all cores)."""
    hd = C // H
    assert hd == 64 and H % 2 == 0 and C % P == 0 and T % QW == 0
    CT = C // P            # contraction tiles over C (6)
    TT = T // P            # t tiles (8)
    NQC = T // QW          # q chunks (2)
    NHP = H // 2           # head pairs (6)
    NG = NHP // 2          # attention groups of 2 head-pair streams (3)
    scale = 1.0 / float(np.sqrt(hd))

    nc = bacc.Bacc("TRN2", target_bir_lowering=False, debug=False,
                   num_devices=n_cores)

    # host-packed inputs: per-partition-contiguous blocks in use order
    #  xd: [p, 3 ct-pair blocks, 2 ct, T]
    #  wd: [p, 9 blocks, ct, 256] with blocks
    #      [q01, k01, v0, q23, k23, v1, q45, k45, v2]
    xd = nc.dram_tensor("xd", [P, CT, T], BF16, kind="ExternalInput").ap()
    wd = nc.dram_tensor("wd", [P, 9, CT, 256], BF16,
                        kind="ExternalInput").ap()
    bqk = nc.dram_tensor("bqk", [P, 2 * CT], F32, kind="ExternalInput").ap()
    bvr = nc.dram_tensor("bvr", [1, C], BF16, kind="ExternalInput").ap()
    yT = nc.dram_tensor("yT", [C, T], F32, kind="ExternalOutput").ap()

    with tile.TileContext(nc) as tc, ExitStack() as ctx:
        const = ctx.enter_context(tc.tile_pool(name="const", bufs=1))
        # PSUM: "s" 4 banks (score tiles), "f" 2 banks (qk/v chains),
        # "y" 2 banks (attention accumulators) = 8 banks total
        spsum = ctx.enter_context(tc.tile_pool(name="spsum", bufs=2, space="PSUM"))
        fpsum = ctx.enter_context(tc.tile_pool(name="fpsum", bufs=2, space="PSUM"))
        ypsum = ctx.enter_context(tc.tile_pool(name="ypsum", bufs=2, space="PSUM"))
        scb = ctx.enter_context(tc.tile_pool(name="scb", bufs=14))
        ysb = ctx.enter_context(tc.tile_pool(name="ysb", bufs=3))

        wt_sb = const.tile([P, 9, CT, 256], BF16)
        xt_sb = const.tile([P, CT, T], BF16)
        bqk_sb = const.tile([P, 2 * CT], F32)
        bvr_sb = const.tile([1, C], BF16)
        ones_sb = const.tile([1, P], BF16)
        qkT = const.tile([P, 2 * CT, T], BF16)   # o-tiles: q = 0..5, k = 6..11
        vsb = const.tile([P, TT, C], BF16)       # v in natural [t, o] layout
        mask_sb = const.tile([P, QW], BF16)

        # ---- input DMA issue (ring order == consumption order) ----
        # Split x per-ct across both rings; first W blocks lead on sync.
        # Remaining W blocks are interleaved between ACT evictions below.
        nc.sync.dma_start(wt_sb[:, 0], wd[:, 0])     # Wq01
        nc.scalar.dma_start(bqk_sb[:], bqk[:])
        nc.scalar.dma_start(bvr_sb[:], bvr[:])
        for ct in range(CT):
            eng = nc.sync if ct % 2 == 0 else nc.scalar
            eng.dma_start(xt_sb[:, ct, :], xd[:, ct])
        nc.sync.dma_start(wt_sb[:, 1], wd[:, 1])     # Wk01
        w_dma = deque(range(2, 9))                   # v0, q23, k23, v1, ...

        def issue_w():
            if w_dma:
                b = w_dma.popleft()
                nc.scalar.dma_start(wt_sb[:, b], wd[:, b])

        # 0/1 upper-triangle mask const (also used as warm-up operand)
        nc.gpsimd.memset(ones_sb[:], 1.0)
        nc.gpsimd.memset(mask_sb[:], 1.0)
        nc.gpsimd.affine_select(
            mask_sb[:], mask_sb[:], pattern=[[1, QW]],
            compare_op=ALU.is_ge, fill=0.0, base=0, channel_multiplier=-1)

        # ---- PE warm-up: keep HAM busy while first inputs stream in ----
        warm_ps = ypsum.tile([P, QW], F32, tag="y", name="warm")
        for _ in range(14):
            nc.tensor.matmul(warm_ps[:], mask_sb[:, 0:P], mask_sb[:],
                             start=True, stop=True)

        evict = [0]

        def emit_qk(ot):
            # ot 0..5 = q features (head pair = ot), 6..11 = k features;
            # one 1-bank psum tile + one ACT bias-evict per q chunk
            j = ot if ot < CT else ot - CT
            blk = 3 * (j // 2) + (0 if ot < CT else 1)
            off = (j % 2) * P
            for qc in range(NQC):
                ps = fpsum.tile([P, QW], F32, tag="f", name="qk_ps")
                for ct in range(CT):
                    nc.tensor.matmul(
                        ps[:],
                        wt_sb[:, blk, ct, off:off + P],
                        xt_sb[:, ct, qc * QW:(qc + 1) * QW],
                        start=(ct == 0), stop=(ct == CT - 1),
                    )
                if qc == 0:
                    issue_w()
                nc.scalar.activation(
                    qkT[:, ot, qc * QW:(qc + 1) * QW], ps[:],
                    AF.Identity, bias=bqk_sb[:, ot:ot + 1])

        def emit_v(s, tp):
            # v features s*256..(s+1)*256 for t-tiles 2tp, 2tp+1; the bias
            # lands via a final K=1 rank-1 matmul (ones^T @ bias_row)
            ps = fpsum.tile([P, QW], F32, tag="f", name="v_ps")
            for j in range(2):
                tt = 2 * tp + j
                for ct in range(CT):
                    nc.tensor.matmul(
                        ps[:, j * 256:(j + 1) * 256],
                        xt_sb[:, ct, tt * P:(tt + 1) * P],
                        wt_sb[:, 3 * s + 2, ct, :],
                        start=(ct == 0), stop=False,
                    )
                nc.tensor.matmul(
                    ps[:, j * 256:(j + 1) * 256], ones_sb[:],
                    bvr_sb[:, s * 256:(s + 1) * 256],
                    start=False, stop=True,
                )
            dst = vsb[:, 2 * tp:2 * tp + 2, s * 256:(s + 1) * 256]
            src = ps.rearrange("p (a b) -> p a b", a=2)
            if evict[0] % 2 == 0:
                nc.scalar.activation(dst, src, AF.Copy)
            else:
                nc.vector.tensor_copy(dst, src)
            evict[0] += 1

        def relu_evict(dst, src):
            # relu(scale * s): PSUM -> SBUF bf16, alternating ACT / DVE per
            # item (one instruction per item keeps the av pair in sync)
            if evict[0] % 2 == 0:
                nc.scalar.activation(dst, src, AF.Relu, scale=scale)
            else:
                nc.vector.tensor_scalar(dst, src, scale, 0.0, ALU.mult, ALU.max)
            evict[0] += 1

        def attention_closures(hp):
            """Per-item (scores, att@v) emission closures for one head pair;
            interleaver runs att@v LAG items behind scores."""
            items = []
            for qc in range(NQC):
                kb_hi = min((qc * QW + QW - 1) // P, TT - 1)
                for kb in range(kb_hi + 1):
                    items.append((qc, kb, kb_hi))
            state = {"s": {}, "y": {}}
            sc_fns, av_fns = [], []

            def sc(i, qc, kb, kb_hi):
                delta = max(kb * P - qc * QW, 0)   # first valid t_q col
                sp = spsum.tile([P, 2, QW], F32, tag="s", name="s_ps")
                for h, ppos in ((0, (0, 0)), (1, (64, 0))):
                    nc.tensor.matmul(
                        sp[:, h, delta:QW],
                        qkT[h * 64:(h + 1) * 64, CT + hp,
                            kb * P:(kb + 1) * P],
                        qkT[h * 64:(h + 1) * 64, hp,
                            qc * QW + delta:(qc + 1) * QW],
                        start=True, stop=True, tile_position=ppos,
                    )
                s = scb.tile([P, 2, QW], BF16, tag="s")
                relu_evict(s[:, :, delta:QW], sp[:, :, delta:QW])
                if kb * P >= qc * QW:   # diagonal block: causal mask on the
                    # first P cols only (row p can only mask j' < p < P)
                    nc.gpsimd.affine_select(
                        s[:, :, delta:delta + P],
                        s[:, :, delta:delta + P],
                        pattern=[[0, 2], [1, P]],
                        compare_op=ALU.is_ge, fill=0.0,
                        base=0, channel_multiplier=-1,
                    )
                state["s"][i] = s

            def av(i, qc, kb, kb_hi):
                if kb == 0:
                    state["y"][qc] = ypsum.tile([P, QW], F32, tag="y",
                                                name="yp")
                yp = state["y"][qc]
                delta = max(kb * P - qc * QW, 0)
                s = state["s"].pop(i)
                # two heads accumulate into disjoint partition ranges of one
                # bank; each runs its own start/stop group
                nc.tensor.matmul(
                    yp[0:64, delta:QW], vsb[:, kb, hp * P:hp * P + 64],
                    s[:, 0, delta:QW],
                    start=(kb == 0), stop=(kb == kb_hi),
                    tile_position=(0, 0), skip_group_check=True,
                )
                nc.tensor.matmul(
                    yp[64:128, delta:QW],
                    vsb[:, kb, hp * P + 64:hp * P + 128],
                    s[:, 1, delta:QW],
                    start=(kb == 0), stop=(kb == kb_hi),
                    tile_position=(0, 64), skip_group_check=True,
                )
                if kb == kb_hi:
                    yp = state["y"].pop(qc)
                    yt = ysb.tile([P, QW], F32, tag="yt")
                    nc.vector.tensor_copy(yt[:], yp[:])
                    nc.sync.dma_start(
                        yT[hp * P:(hp + 1) * P, qc * QW:(qc + 1) * QW],
                        yt[:])

            for i, (qc, kb, kb_hi) in enumerate(items):
                sc_fns.append(
                    lambda i=i, qc=qc, kb=kb, kb_hi=kb_hi: sc(i, qc, kb, kb_hi))
                av_fns.append(
                    lambda i=i, qc=qc, kb=kb, kb_hi=kb_hi: av(i, qc, kb, kb_hi))
            return sc_fns, av_fns

        def group_fillers(g):
            """qk + v chain thunks needed by attention group g."""
            fns = []
            for hp in (2 * g, 2 * g + 1):
                fns.append(lambda ot=hp: emit_qk(ot))
                fns.append(lambda ot=CT + hp: emit_qk(ot))
            for tp in range(TT // 2):
                fns.append(lambda s=g, tp=tp: emit_v(s, tp))
            return fns

        # ---- fused emission ----
        # pre-phase: everything group 0 needs
        for fn in group_fillers(0):
            fn()

        # Cross-group pipeline: each group's tail att@v steps interleave
        # with the next group's first score steps so the PE never dips at
        # group boundaries (a dip re-throttles the HAM clock gate).
        LAG = 3
        pending = []     # av emission thunk-lists carried from prev group
        for g in range(NG):
            streams = [attention_closures(hp) for hp in (2 * g, 2 * g + 1)]
            fillers = deque(group_fillers(g + 1) if g + 1 < NG else [])
            n = len(streams[0][0])
            for i in range(n):
                for sc_fns, _ in streams:
                    sc_fns[i]()
                if i < LAG:
                    if pending:
                        for fn in pending.pop(0):
                            fn()
                else:
                    for _, av_fns in streams:
                        av_fns[i - LAG]()
                if fillers:
                    fillers.popleft()()
            while fillers:
                fillers.popleft()()
            pending = [[av_fns[j] for _, av_fns in streams]
                       for j in range(n - LAG, n)]
        for tail in pending:
            for fn in tail:
                fn()

    nc.compile()
    return nc


_CACHE = {}


def _get_module():
    if "nc" not in _CACHE:
        _CACHE["nc"] = build_module()
    return _CACHE["nc"]


def _prep_in_maps(x, W_attn, b_attn, T=1024, C=768, n_cores=8):
    bf = ml_dtypes.bfloat16
    CT = C // P
    WT = np.ascontiguousarray(W_attn.astype(np.float32).T)     # [C, 3C]
    # [C, 3C] -> [p, ct, 3C]
    W3 = WT.reshape(CT, P, 3 * C).transpose(1, 0, 2)
    # 9 blocks of 256 o-cols in use order [q01,k01,v0,q23,k23,v1,q45,k45,v2]
    blocks = []
    for gg in range(3):
        blocks.append(W3[:, :, gg * 256:(gg + 1) * 256])             # q
        blocks.append(W3[:, :, C + gg * 256:C + (gg + 1) * 256])     # k
        blocks.append(W3[:, :, 2 * C + gg * 256:2 * C + (gg + 1) * 256])  # v
    wd = np.ascontiguousarray(
        np.stack(blocks, axis=1)).astype(bf)                   # [p, 9, ct, 256]
    bqk = np.ascontiguousarray(
        b_attn[:2 * C].astype(np.float32).reshape(2 * CT, P).T)  # [P, 12]
    bvr = np.ascontiguousarray(
        b_attn[2 * C:].astype(np.float32)[None, :]).astype(bf)  # [1, C]
    in_maps = []
    for c in range(n_cores):
        xT_b = np.ascontiguousarray(x[c].astype(np.float32).T)  # [C, T]
        xdb = np.ascontiguousarray(
            xT_b.reshape(CT, P, T).transpose(1, 0, 2)).astype(bf)
        in_maps.append({"xd": xdb, "wd": wd, "bqk": bqk, "bvr": bvr})
    return in_maps


def run(x, W_attn, b_attn, trace=False):
    nc = _get_module()
    in_maps = _prep_in_maps(x, W_attn, b_attn)
    res = bass_utils.run_bass_kernel_spmd(
        nc, in_maps, core_ids=list(range(8)), trace=trace)
    y = np.stack([np.asarray(res.results[c]["yT"]).T for c in range(8)])
    return np.ascontiguousarray(y.astype(np.float32)), res


def kernel(x, W_attn, b_attn):
    y, _ = run(x, W_attn, b_attn, trace=False)
    return y


# revision 26
# speedup vs baseline: 1.0667x; 1.0269x over previous
"""Causal ReLU-attention block (qkv proj + per-head attention) on 8 trn2 cores.

Sharding: pure data-parallel over batch (B=8 -> 1 batch element per core).

Per-core structure (fused single pass, PE-stream order):
  warm-up MMs -> qk ots for head-pair group 0 -> v slice 0 ->
  [attention group 0 interleaved with qk/v chains for group 1] ->
  [attention group 1 interleaved with qk/v chains for group 2] ->
  attention group 2.

DMA: host packs x/W into per-partition-contiguous blocks ordered by first
use; sync ring carries x (+ y out), scalar ring carries W/biases, with W
issue instructions interleaved between ACT evictions.
"""

import sys
from collections import deque
from contextlib import ExitStack

sys.path.insert(0, "/opt/trn_rl_repo")

import ml_dtypes
import numpy as np

import concourse.bass as bass
import concourse.tile as tile
from concourse import bacc, bass_utils, mybir

P = 128
QW = 512  # t_q chunk width (PSUM bank = 512 fp32)

BF16 = mybir.dt.bfloat16
F32 = mybir.dt.float32
AF = mybir.ActivationFunctionType
ALU = mybir.AluOpType


def build_module(T=1024, C=768, H=12, n_cores=8):
    """Build + compile the per-core Bass module (same program on all cores)."""
    hd = C // H
    assert hd == 64 and H % 2 == 0 and C % P == 0 and T % QW == 0
    CT = C // P            # contraction tiles over C (6)
    TT = T // P            # t tiles (8)
    NQC = T // QW          # q chunks (2)
    NHP = H // 2           # head pairs (6)
    NG = NHP // 2          # attention groups of 2 head-pair streams (3)
    scale = 1.0 / float(np.sqrt(hd))

    nc = bacc.Bacc("TRN2", target_bir_lowering=False, debug=False,
                   num_devices=n_cores)

    # host-packed inputs: per-partition-contiguous blocks in use order
    #  xd: [p, 3 ct-pair blocks, 2 ct, T]
    #  wd: [p, 9 blocks, ct, 256] with blocks
    #      [q01, k01, v0, q23, k23, v1, q45, k45, v2]
    xd = nc.dram_tensor("xd", [P, CT, T], BF16, kind="ExternalInput").ap()
    wd = nc.dram_tensor("wd", [P, 9, CT, 256], BF16,
                        kind="ExternalInput").ap()
    bqk = nc.dram_tensor("bqk", [P, 2 * CT], F32, kind="ExternalInput").ap()
    bvr = nc.dram_tensor("bvr", [1, C], BF16, kind="ExternalInput").ap()
    yT = nc.dram_tensor("yT", [C, T], F32, kind="ExternalOutput").ap()

    with tile.TileContext(nc) as tc, ExitStack() as ctx:
        const = ctx.enter_context(tc.tile_pool(name="const", bufs=1))
        # PSUM: "s" 4 banks (score tiles), "f" 2 banks (qk/v chains),
        # "y" 2 banks (attention accumulators) = 8 banks total
        spsum = ctx.enter_context(tc.tile_pool(name="spsum", bufs=2, space="PSUM"))
        fpsum = ctx.enter_context(tc.tile_pool(name="fpsum", bufs=2, space="PSUM"))
        ypsum = ctx.enter_context(tc.tile_pool(name="ypsum", bufs=2, space="PSUM"))
        scb = ctx.enter_context(tc.tile_pool(name="scb", bufs=14))
        ysb = ctx.enter_context(tc.tile_pool(name="ysb", bufs=3))

        wt_sb = const.tile([P, 9, CT, 256], BF16)
        xt_sb = const.tile([P, CT, T], BF16)
        bqk_sb = const.tile([P, 2 * CT], F32)
        bvr_sb = const.tile([1, C], BF16)
        ones_sb = const.tile([1, P], BF16)
        qkT = const.tile([P, 2 * CT, T], BF16)   # o-tiles: q = 0..5, k = 6..11
        vsb = const.tile([P, TT, C], BF16)       # v in natural [t, o] layout
        mask_sb = const.tile([P, QW], BF16)

        # ---- input DMA issue (ring order == consumption order) ----
        # Split x per-ct across both rings; first W blocks lead on sync.
        # Remaining W blocks are interleaved between ACT evictions below.
        nc.sync.dma_start(wt_sb[:, 0], wd[:, 0])     # Wq01
        nc.scalar.dma_start(bqk_sb[:], bqk[:])
        nc.scalar.dma_start(bvr_sb[:], bvr[:])
        for ct in range(CT):
            eng = nc.sync if ct % 2 == 0 else nc.scalar
            eng.dma_start(xt_sb[:, ct, :], xd[:, ct])
        nc.sync.dma_start(wt_sb[:, 1], wd[:, 1])     # Wk01
        w_dma = deque(range(2, 9))                   # v0, q23, k23, v1, ...

        def issue_w():
            if w_dma:
                b = w_dma.popleft()
                nc.scalar.dma_start(wt_sb[:, b], wd[:, b])

        # 0/1 upper-triangle mask const (also used as warm-up operand)
        nc.gpsimd.memset(ones_sb[:], 1.0)
        nc.gpsimd.memset(mask_sb[:], 1.0)
        nc.gpsimd.affine_select(
            mask_sb[:], mask_sb[:], pattern=[[1, QW]],
            compare_op=ALU.is_ge, fill=0.0, base=0, channel_multiplier=-1)

        # ---- PE warm-up: keep HAM busy while first inputs stream in ----
        warm_ps = ypsum.tile([P, QW], F32, tag="y", name="warm")
        for _ in range(14):
            nc.tensor.matmul(warm_ps[:], mask_sb[:, 0:P], mask_sb[:],
                             start=True, stop=True)

        evict = [0]

        def emit_qk(ot):
            # ot 0..5 = q features (head pair = ot), 6..11 = k features;
            # one 1-bank psum tile + one ACT bias-evict per q chunk
            j = ot if ot < CT else ot - CT
            blk = 3 * (j // 2) + (0 if ot < CT else 1)
            off = (j % 2) * P
            for qc in range(NQC):
                ps = fpsum.tile([P, QW], F32, tag="f", name="qk_ps")
                for ct in range(CT):
                    nc.tensor.matmul(
                        ps[:],
                        wt_sb[:, blk, ct, off:off + P],
                        xt_sb[:, ct, qc * QW:(qc + 1) * QW],
                        start=(ct == 0), stop=(ct == CT - 1),
                    )
                if qc == 0:
                    issue_w()
                nc.scalar.activation(
                    qkT[:, ot, qc * QW:(qc + 1) * QW], ps[:],
                    AF.Identity, bias=bqk_sb[:, ot:ot + 1])

        def emit_v(s, tp):
            # v features s*256..(s+1)*256 for t-tiles 2tp, 2tp+1; the bias
            # lands via a final K=1 rank-1 matmul (ones^T @ bias_row)
            ps = fpsum.tile([P, QW], F32, tag="f", name="v_ps")
            for j in range(2):
                tt = 2 * tp + j
                for ct in range(CT):
                    nc.tensor.matmul(
                        ps[:, j * 256:(j + 1) * 256],
                        xt_sb[:, ct, tt * P:(tt + 1) * P],
                        wt_sb[:, 3 * s + 2, ct, :],
                        start=(ct == 0), stop=False,
                    )
                nc.tensor.matmul(
                    ps[:, j * 256:(j + 1) * 256], ones_sb[:],
                    bvr_sb[:, s * 256:(s + 1) * 256],
                    start=False, stop=True,
                )
            dst = vsb[:, 2 * tp:2 * tp + 2, s * 256:(s + 1) * 256]
            src = ps.rearrange("p (a b) -> p a b", a=2)
            if evict[0] % 2 == 0:
                nc.scalar.activation(dst, src, AF.Copy)
            else:
                nc.vector.tensor_copy(dst, src)
            evict[0] += 1

        def relu_evict(dst, src):
            # relu(scale * s): PSUM -> SBUF bf16, alternating ACT / DVE per
            # item (one instruction per item keeps the av pair in sync)
            if evict[0] % 2 == 0:
                nc.scalar.activation(dst, src, AF.Relu, scale=scale)
            else:
                nc.vector.tensor_scalar(dst, src, scale, 0.0, ALU.mult, ALU.max)
            evict[0] += 1

        def attention_closures(hp):
            """Per-item (scores, att@v) emission closures for one head pair;
            interleaver runs att@v LAG items behind scores."""
            items = []
            for qc in range(NQC):
                kb_hi = min((qc * QW + QW - 1) // P, TT - 1)
                for kb in range(kb_hi + 1):
                    items.append((qc, kb, kb_hi))
            state = {"s": {}, "y": {}}
            sc_fns, av_fns = [], []

            def sc(i, qc, kb, kb_hi):
                delta = max(kb * P - qc * QW, 0)   # first valid t_q col
                sp = spsum.tile([P, 2, QW], F32, tag="s", name="s_ps")
                for h, ppos in ((0, (0, 0)), (1, (64, 0))):
                    nc.tensor.matmul(
                        sp[:, h, delta:QW],
                        qkT[h * 64:(h + 1) * 64, CT + hp,
                            kb * P:(kb + 1) * P],
                        qkT[h * 64:(h + 1) * 64, hp,
                            qc * QW + delta:(qc + 1) * QW],
                        start=True, stop=True, tile_position=ppos,
                    )
                s = scb.tile([P, 2, QW], BF16, tag="s")
                relu_evict(s[:, :, delta:QW], sp[:, :, delta:QW])
                if kb * P >= qc * QW:   # diagonal block: causal mask on the
                    # first P cols only (row p can only mask j' < p < P)
                    nc.gpsimd.affine_select(
                        s[:, :, delta:delta + P],
                        s[:, :, delta:delta + P],
                        pattern=[[0, 2], [1, P]],
                        compare_op=ALU.is_ge, fill=0.0,
                        base=0, channel_multiplier=-1,
                    )
                state["s"][i] = s

            def av(i, qc, kb, kb_hi):
                if kb == 0:
                    state["y"][qc] = ypsum.tile([P, QW], F32, tag="y",
                                                name="yp")
                yp = state["y"][qc]
                delta = max(kb * P - qc * QW, 0)
                s = state["s"].pop(i)
                # two heads accumulate into disjoint partition ranges of one
                # bank; each runs its own start/stop group
                nc.tensor.matmul(
                    yp[0:64, delta:QW], vsb[:, kb, hp * P:hp * P + 64],
                    s[:, 0, delta:QW],
                    start=(kb == 0), stop=(kb == kb_hi),
                    tile_position=(0, 0), skip_group_check=True,
                )
                nc.tensor.matmul(
                    yp[64:128, delta:QW],
                    vsb[:, kb, hp * P + 64:hp * P + 128],
                    s[:, 1, delta:QW],
                    start=(kb == 0), stop=(kb == kb_hi),
                    tile_position=(0, 64), skip_group_check=True,
                )
                if kb == kb_hi:
                    yp = state["y"].pop(qc)
                    yt = ysb.tile([P, QW], F32, tag="yt")
                    nc.vector.tensor_copy(yt[:], yp[:])
                    nc.sync.dma_start(
                        yT[hp * P:(hp + 1) * P, qc * QW:(qc + 1) * QW],
                        yt[:])

            for i, (qc, kb, kb_hi) in enumerate(items):
                sc_fns.append(
                    lambda i=i, qc=qc, kb=kb, kb_hi=kb_hi: sc(i, qc, kb, kb_hi))
                av_fns.append(
                    lambda i=i, qc=qc, kb=kb, kb_hi=kb_hi: av(i, qc, kb, kb_hi))
            return sc_fns, av_fns

        def group_fillers(g):
            """Fillers emitted during group g's attention: g's own v slices
            (consumed by g's lagged att@v steps) + qk for group g+1."""
            fns = [lambda s=g, tp=tp: emit_v(s, tp) for tp in range(TT // 2)]
            if g + 1 < NG:
                for hp in (2 * (g + 1), 2 * (g + 1) + 1):
                    fns.append(lambda ot=hp: emit_qk(ot))
                    fns.append(lambda ot=CT + hp: emit_qk(ot))
            return fns

        # ---- fused emission ----
        # pre-phase: group 0's q/k projections only
        for hp in (0, 1):
            emit_qk(hp)
            emit_qk(CT + hp)

        # Cross-group pipeline: each group's tail att@v steps interleave
        # with the next group's first score steps so the PE never dips at
        # group boundaries (a dip re-throttles the HAM clock gate).
        LAG = 3
        pending = []     # av emission thunk-lists carried from prev group
        for g in range(NG):
            streams = [attention_closures(hp) for hp in (2 * g, 2 * g + 1)]
            fillers = deque(group_fillers(g))
            n = len(streams[0][0])
            for i in range(n):
                for sc_fns, _ in streams:
                    sc_fns[i]()
                if i < LAG:
                    if pending:
                        for fn in pending.pop(0):
                            fn()
                else:
                    for _, av_fns in streams:
                        av_fns[i - LAG]()
                if fillers:
                    fillers.popleft()()
            while fillers:
                fillers.popleft()()
            pending = [[av_fns[j] for _, av_fns in streams]
                       for j in range(n - LAG, n)]
        for tail in pending:
            for fn in tail:
                fn()

    nc.compile()
    return nc


_CACHE = {}


def _get_module():
    if "nc" not in _CACHE:
        _CACHE["nc"] = build_module()
    return _CACHE["nc"]


def _prep_in_maps(x, W_attn, b_attn, T=1024, C=768, n_cores=8):
    bf = ml_dtypes.bfloat16
    CT = C // P
    WT = np.ascontiguousarray(W_attn.astype(np.float32).T)     # [C, 3C]
    # [C, 3C] -> [p, ct, 3C]
    W3 = WT.reshape(CT, P, 3 * C).transpose(1, 0, 2)
    # 9 blocks of 256 o-cols in use order [q01,k01,v0,q23,k23,v1,q45,k45,v2]
    blocks = []
    for gg in range(3):
        blocks.append(W3[:, :, gg * 256:(gg + 1) * 256])             # q
        blocks.append(W3[:, :, C + gg * 256:C + (gg + 1) * 256])     # k
        blocks.append(W3[:, :, 2 * C + gg * 256:2 * C + (gg + 1) * 256])  # v
    wd = np.ascontiguousarray(
        np.stack(blocks, axis=1)).astype(bf)                   # [p, 9, ct, 256]
    bqk = np.ascontiguousarray(
        b_attn[:2 * C].astype(np.float32).reshape(2 * CT, P).T)  # [P, 12]
    bvr = np.ascontiguousarray(
        b_attn[2 * C:].astype(np.float32)[None, :]).astype(bf)  # [1, C]
    in_maps = []
    for c in range(n_cores):
        xT_b = np.ascontiguousarray(x[c].astype(np.float32).T)  # [C, T]
        xdb = np.ascontiguousarray(
            xT_b.reshape(CT, P, T).transpose(1, 0, 2)).astype(bf)
        in_maps.append({"xd": xdb, "wd": wd, "bqk": bqk, "bvr": bvr})
    return in_maps


def run(x, W_attn, b_attn, trace=False):
    nc = _get_module()
    in_maps = _prep_in_maps(x, W_attn, b_attn)
    res = bass_utils.run_bass_kernel_spmd(
        nc, in_maps, core_ids=list(range(8)), trace=trace)
    y = np.stack([np.asarray(res.results[c]["yT"]).T for c in range(8)])
    return np.ascontiguousarray(y.astype(np.float32)), res


def kernel(x, W_attn, b_attn):
    y, _ = run(x, W_attn, b_attn, trace=False)
    return y
